# revision 1
# baseline (speedup 1.0000x reference)
"""Trainium2 Bass kernel for nn_ComplexMultiheadAttention.

Problem (reference.py): complex multihead attention,
  B=2, N=1024, D=1024, HEADS=16, dim_head=64.
  q/k/v = complex linear projections of x = x_real + i*x_imag,
  4 softmax-attention combos g0..g3 over (q-part, k-part, v-part),
  sign-combined into o_real/o_imag, then a complex output projection.
  Output: [2, B, N, D] fp32 (real, imag).

Sharding (8 NeuronCores): core c = (b = c // 4) x (head group hg = c % 4,
4 heads each). Each core computes projections + attention + sign-combine
for its 4 heads and a partial output projection (its heads' contribution,
full output columns); the host unshards by summing the 4 partials per
batch.

Key kernel choices:
- All matmuls in float32r (full PE rate at moving dim >= 256; ~1e-3 rel err).
- Complex linears fused: X = [x_r^T; x_i^T] (K=2048) against host-prepared
  sign-combined weight stacks, so each complex output needs one PSUM chain.
- q/k computed transposed (q^T = [d, tokens]) so QK^T runs as S^T = K Q^T
  with head-PAIR row-packing (two K=64 matmuls per PE pass via
  tile_position (0,0)/(64,0)).
- softmax without max-subtraction (|S*scale| < ~6 by construction), exp on
  ScalarE straight out of PSUM, denominator via a ones-row appended to V
  (AV output row 64 = sum_j P^T[j,i]), reciprocal broadcast across
  partitions with a tiny SBUF->SBUF DMA.
- Projection weights streamed per k-tile (SBUF is tight: X alone is 64KB/p).
"""

import numpy as np

import concourse.mybir as mybir
import concourse.tile as tile
from concourse import bacc
from concourse.bass_utils import run_bass_kernel_spmd

P = 128
NTOK = 1024  # tokens per batch
KD = 16  # k-tiles over the stacked 2048 contraction dim
CD = 64  # dim per head
HL = 4  # heads per core
F32 = mybir.dt.float32
F32R = mybir.dt.float32r
BF16 = mybir.dt.bfloat16
EXP = mybir.ActivationFunctionType.Exp
SCALE = float(CD) ** -0.5

_nc_cache = None


def _build():
    nc = bacc.Bacc("TRN2", target_bir_lowering=False, debug=False, num_devices=8)

    x = nc.declare_dram_parameter("x", [2048, NTOK], F32, isOutput=False)
    wnames = ["wqr", "wqi", "wkr", "wki"]
    wd = {n: nc.declare_dram_parameter(n, [2048, 256], F32, isOutput=False) for n in wnames}
    wv = nc.declare_dram_parameter("wv", [2048, 512], F32, isOutput=False)
    wyr = nc.declare_dram_parameter("wyr", [512, NTOK], F32, isOutput=False)
    wyi = nc.declare_dram_parameter("wyi", [512, NTOK], F32, isOutput=False)
    yp = nc.declare_dram_parameter("ypart", [2, NTOK, 1024], F32, isOutput=True)

    with tile.TileContext(nc) as tc:
        with (
            tc.tile_pool(name="persist", bufs=1) as pp,
            tc.tile_pool(name="small", bufs=2) as sp,
        ):
            # q^T/k^T duplicated along partitions: [128 = head d(64) twice,
            # head, tokens]. K=128 QK matmuls compute 2*S and keep the PE's
            # HAM activity monitor armed (K=64 matmuls leave it throttled at
            # 1.2 GHz); the extra factor 2 is folded into the exp scale.
            qrT = pp.tile([P, HL, NTOK], BF16, tag="qrT")
            qiT = pp.tile([P, HL, NTOK], BF16, tag="qiT")
            krT = pp.tile([P, HL, NTOK], BF16, tag="krT")
            kiT = pp.tile([P, HL, NTOK], BF16, tag="kiT")
            # V with ones column appended: [tok-tile, jt, head, 65]
            vhat_r = pp.tile([P, 8, HL, CD + 1], BF16, tag="vhr")
            vhat_i = pp.tile([P, 8, HL, CD + 1], BF16, tag="vhi")
            # combined attention output, kt-blocks of [o_r_h(64); o_i_h(64)]
            O = pp.tile([P, HL, NTOK], F32R, tag="O")

            # ---------------- Phase A: projections ----------------
            with (
                tc.tile_pool(name="pa", bufs=1) as pa,
                tc.tile_pool(name="pa_w", bufs=6) as paw,
                tc.tile_pool(name="pa_ps", bufs=8, space="PSUM") as paps,
            ):
                xs = pa.tile([P, KD, NTOK], F32R, tag="xs")
                xt = x.rearrange("(o p) m -> p o m", p=P).bitcast(F32R)
                for kt in range(KD):
                    nc.sync.dma_start(xs[:, kt, :], xt[:, kt, :])
                wtiled = {
                    n: wd[n].rearrange("(o p) m -> p o m", p=P).bitcast(F32R)
                    for n in wnames
                }
                wvt = wv.rearrange("(o p) m -> p o m", p=P).bitcast(F32R)

                def emit_v():
                    # fused v chain: rhs = [Wvr | Wvi] (N=512), one chain for
                    # both complex parts
                    wt = paw.tile([P, KD, 512], F32R, tag="wtv", name="wt", bufs=1)
                    for kt in range(KD):
                        nc.sync.dma_start(wt[:, kt, :], wvt[:, kt, :])
                    for half in range(2):
                        pss = [
                            paps.tile([P, 512], F32, tag="proj", name="ps")
                            for _ in range(4)
                        ]
                        for kt in range(KD):
                            for j in range(4):
                                tt = half * 4 + j
                                nc.tensor.matmul(
                                    pss[j][:],
                                    xs[:, kt, tt * 128 : (tt + 1) * 128],
                                    wt[:, kt, :],
                                    start=(kt == 0),
                                    stop=(kt == KD - 1),
                                )
                        for j in range(4):
                            tt = half * 4 + j
                            nc.vector.tensor_copy(
                                vhat_r[:, tt, :, 0:CD],
                                pss[j][:, 0:256].rearrange("p (h d) -> p h d", d=CD),
                            )
                            nc.vector.tensor_copy(
                                vhat_i[:, tt, :, 0:CD],
                                pss[j][:, 256:512].rearrange("p (h d) -> p h d", d=CD),
                            )
                # q^T, k^T per pair (pair-major so heads 0/1 finish early and
                # attention overlaps the tail of the projections); v chains go
                # right after pair 0 so AV for heads 0/1 unblocks early
                def emit_qk_pair(pair):
                    for wn, dstT in (
                        ("wqr", qrT),
                        ("wqi", qiT),
                        ("wkr", krT),
                        ("wki", kiT),
                    ):
                        pss = [
                            paps.tile([P, 512], F32, tag="proj", name="ps")
                            for _ in range(2)
                        ]
                        wt = paw.tile([P, KD, 128], F32R, tag="wt2", name="wt", bufs=2)
                        for k0 in range(0, KD, 4):
                            nc.sync.dma_start(
                                wt[:, k0 : k0 + 4, :],
                                wtiled[wn][
                                    :, k0 : k0 + 4, pair * 128 : (pair + 1) * 128
                                ],
                            )
                        for kt in range(KD):
                            for tch in range(2):
                                nc.tensor.matmul(
                                    pss[tch][:],
                                    wt[:, kt, :],
                                    xs[:, kt, tch * 512 : (tch + 1) * 512],
                                    start=(kt == 0),
                                    stop=(kt == KD - 1),
                                )
                        for tch in range(2):
                            ps = pss[tch]
                            sl = slice(tch * 512, (tch + 1) * 512)
                            # head A rows 0:64 stay low, head B rows 64:128
                            # high; missing halves DMA-shifted in below.
                            nc.vector.tensor_copy(
                                dstT[0:CD, pair * 2, sl], ps[0:CD, :]
                            )
                            nc.vector.tensor_copy(
                                dstT[CD:P, pair * 2 + 1, sl], ps[CD:P, :]
                            )
                        for h in (pair * 2, pair * 2 + 1):
                            if h % 2 == 0:
                                nc.gpsimd.dma_start(
                                    dstT[CD:P, h, :], dstT[0:CD, h, :]
                                )
                            else:
                                nc.gpsimd.dma_start(
                                    dstT[0:CD, h, :], dstT[CD:P, h, :]
                                )
                emit_qk_pair(0)
                emit_v()
                emit_qk_pair(1)
                ones = pa.tile([P, 8, HL, 1], F32, tag="ones")
                nc.vector.memset(ones[:], 1.0)
                nc.vector.tensor_copy(vhat_r[:, :, :, CD : CD + 1], ones[:])
                nc.vector.tensor_copy(vhat_i[:, :, :, CD : CD + 1], ones[:])

            # ---------------- Phase B: attention ----------------
            # Software-pipelined over the 16 (pair, g, ic) units: QK+exp of
            # unit n is emitted BEFORE AV+normalize of unit n-1, so ScalarE
            # (the pacing engine) is always fed and PE gaps stay below the
            # HAM re-throttle window.
            with (
                tc.tile_pool(name="pb_pt", bufs=3) as ptpool,
                tc.tile_pool(name="pb_on", bufs=1) as onpool,
                tc.tile_pool(name="pb_c", bufs=1) as cpool,
                tc.tile_pool(name="pb_oav", bufs=10) as oavp,
                tc.tile_pool(name="pb_den", bufs=2) as dpool,
                tc.tile_pool(name="pb_s", bufs=3, space="PSUM") as spool,
                tc.tile_pool(name="pb_av", bufs=2, space="PSUM") as avpool,
            ):
                ons = {}

                def get_on(h):
                    if h not in ons:
                        ons[h] = onpool.tile(
                            [CD, 4, NTOK], F32, tag=f"on{h % 2}", name="on"
                        )
                    return ons[h]

                def emit_qk_exp(h, g, ic):
                    qT = qrT if g in (0, 1) else qiT
                    kT = krT if g in (0, 2) else kiT
                    pt = ptpool.tile([P, 8, 512], BF16, tag="pt", name="pt")
                    for u in range(4):
                        st = spool.tile([P, 1024], F32, tag="s", name="st")
                        for jj in range(2):
                            jt = 2 * u + jj
                            nc.tensor.matmul(
                                st[:, jj * 512 : (jj + 1) * 512],
                                kT[:, h, jt * 128 : (jt + 1) * 128],
                                qT[:, h, ic * 512 : (ic + 1) * 512],
                                start=True,
                                stop=True,
                            )
                        nc.scalar.activation(
                            pt[:, 2 * u : 2 * u + 2, :].rearrange("p a b -> p (a b)"),
                            st[:],
                            EXP,
                            scale=SCALE / 2.0,
                        )
                    return pt

                def emit_av(h, g, ic, pt, hstate):
                    vh = vhat_r if g in (0, 2) else vhat_i
                    av = avpool.tile([CD + 1, 512], F32, tag="av", name="av")
                    for jt in range(8):
                        nc.tensor.matmul(
                            av[:],
                            vh[:, jt, h, :],
                            pt[:, jt, :],
                            start=(jt == 0),
                            stop=(jt == 7),
                        )
                    # evacuate PSUM right away so the slot frees fast; the
                    # denominator row is DMA'd into the per-head batch tensor
                    # so ONE [8,512] reciprocal serves the whole head.
                    oav = oavp.tile([CD + 1, 512], F32, tag="oav", name="oav")
                    nc.vector.tensor_copy(oav[:], av[:])
                    iu = g * 2 + ic
                    nc.gpsimd.dma_start(hstate["den"][iu : iu + 1, :], oav[CD : CD + 1, :])
                    hstate["oavs"].append((g, ic, oav))

                def emit_norm_combine(h, hstate):
                    rp8 = sp.tile([8, 512], F32, tag="rp8")
                    nc.vector.reciprocal(rp8[:], hstate["den"][:])
                    on = get_on(h)
                    for g, ic, oav in hstate["oavs"]:
                        iu = g * 2 + ic
                        rp1 = sp.tile([1, 512], F32, tag="rp1")
                        nc.gpsimd.dma_start(rp1[:], rp8[iu : iu + 1, :])
                        bc = sp.tile([CD, 512], F32, tag="bc")
                        nc.gpsimd.partition_broadcast(bc[:], rp1[:])
                        nc.vector.tensor_mul(
                            on[:, g, ic * 512 : (ic + 1) * 512],
                            oav[0:CD, :],
                            bc[:],
                        )
                    emit_combine(h)

                def emit_combine(h):
                    # sign-combine: o_r = (o0-o3)-(o1+o2), o_i = (o0-o3)+(o1+o2)
                    on = ons[h]
                    s = cpool.tile([CD, NTOK], F32, tag="cs", name="cs")
                    t = cpool.tile([CD, NTOK], F32, tag="ct", name="ct")
                    oi = cpool.tile([CD, NTOK], F32R, tag="oi", name="oi")
                    nc.vector.tensor_sub(s[:], on[:, 0, :], on[:, 3, :])
                    nc.vector.tensor_add(t[:], on[:, 1, :], on[:, 2, :])
                    nc.vector.tensor_sub(O[0:CD, h, :], s[:], t[:])
                    nc.vector.tensor_add(oi[:], s[:], t[:])
                    # o_i half lives on partitions 64..127 of O: DMA shift
                    nc.gpsimd.dma_start(O[CD:P, h, :], oi[:])

                units = [
                    (h, g, ic)
                    for h in range(HL)
                    for g in range(4)
                    for ic in range(2)
                ]

                def new_hstate():
                    return {
                        "den": dpool.tile([8, 512], F32, tag="den", name="den"),
                        "oavs": [],
                    }

                hstates = {}
                prev = None
                for unit in units:
                    pt = emit_qk_exp(*unit)
                    if prev is not None:
                        ph = prev[0][0]
                        if ph not in hstates:
                            hstates[ph] = new_hstate()
                        emit_av(*prev[0], prev[1], hstates[ph])
                        if prev[0][1:] == (3, 1):
                            emit_norm_combine(ph, hstates.pop(ph))
                    prev = (unit, pt)
                ph = prev[0][0]
                if ph not in hstates:
                    hstates[ph] = new_hstate()
                emit_av(*prev[0], prev[1], hstates[ph])
                emit_norm_combine(ph, hstates.pop(ph))

            # ---------------- Phase C: output projection ----------------
            with (
                tc.tile_pool(name="pc_w", bufs=1) as wyp,
                tc.tile_pool(name="pc_ps", bufs=8, space="PSUM") as cps,
                tc.tile_pool(name="pc_o", bufs=8) as cop,
            ):
                wyr_sb = wyp.tile([P, HL, NTOK], F32R, tag="wyr")
                wyi_sb = wyp.tile([P, HL, NTOK], F32R, tag="wyi")
                nc.sync.dma_start(
                    wyr_sb[:], wyr.rearrange("(o p) m -> p o m", p=P).bitcast(F32R)
                )
                nc.sync.dma_start(
                    wyi_sb[:], wyi.rearrange("(o p) m -> p o m", p=P).bitcast(F32R)
                )
                for ri, W in ((0, wyr_sb), (1, wyi_sb)):
                    for tt in range(8):
                        for oc in range(2):
                            ps = cps.tile([P, 512], F32, tag="y", name="psy")
                            for kt in range(HL):
                                nc.tensor.matmul(
                                    ps[:],
                                    O[:, kt, tt * 128 : (tt + 1) * 128],
                                    W[:, kt, oc * 512 : (oc + 1) * 512],
                                    start=(kt == 0),
                                    stop=(kt == HL - 1),
                                )
                            ys = cop.tile([P, 512], F32, tag="ys")
                            nc.vector.tensor_copy(ys[:], ps[:])
                            nc.sync.dma_start(
                                yp[
                                    ri,
                                    tt * 128 : (tt + 1) * 128,
                                    oc * 512 : (oc + 1) * 512,
                                ],
                                ys[:],
                            )
    nc.compile()
    return nc


def _prep(inputs):
    f = np.float32
    xr = np.asarray(inputs["x_real"], f)
    xi = np.asarray(inputs["x_imag"], f)
    wq_r = np.asarray(inputs["wq_r"], f)
    wq_i = np.asarray(inputs["wq_i"], f)
    wkv_r = np.asarray(inputs["wkv_r"], f)
    wkv_i = np.asarray(inputs["wkv_i"], f)
    wout_r = np.asarray(inputs["wout_r"], f)
    wout_i = np.asarray(inputs["wout_i"], f)

    c = np.ascontiguousarray
    in_maps = []
    for core in range(8):
        b, hg = divmod(core, 4)
        c0 = hg * 256
        X = np.concatenate([xr[b].T, xi[b].T], axis=0)
        sl = slice(c0, c0 + 256)
        vsl = slice(1024 + c0, 1024 + c0 + 256)
        m = {
            "x": c(X),
            "wqr": c(np.concatenate([wq_r[sl].T, -wq_i[sl].T], axis=0)),
            "wqi": c(np.concatenate([wq_i[sl].T, wq_r[sl].T], axis=0)),
            "wkr": c(np.concatenate([wkv_r[sl].T, -wkv_i[sl].T], axis=0)),
            "wki": c(np.concatenate([wkv_i[sl].T, wkv_r[sl].T], axis=0)),
            "wv": c(
                np.concatenate(
                    [
                        np.concatenate([wkv_r[vsl].T, -wkv_i[vsl].T], axis=0),
                        np.concatenate([wkv_i[vsl].T, wkv_r[vsl].T], axis=0),
                    ],
                    axis=1,
                )
            ),
        }
        Wyr = np.empty((512, 1024), f)
        Wyi = np.empty((512, 1024), f)
        for h in range(HL):
            cols = slice(c0 + h * CD, c0 + (h + 1) * CD)
            Wyr[h * 128 : h * 128 + CD] = wout_r[:, cols].T
            Wyr[h * 128 + CD : (h + 1) * 128] = -wout_i[:, cols].T
            Wyi[h * 128 : h * 128 + CD] = wout_i[:, cols].T
            Wyi[h * 128 + CD : (h + 1) * 128] = wout_r[:, cols].T
        m["wyr"] = c(Wyr)
        m["wyi"] = c(Wyi)
        in_maps.append(m)
    return in_maps


def _get_nc():
    global _nc_cache
    if _nc_cache is None:
        _nc_cache = _build()
    return _nc_cache


def _assemble(results):
    y = np.zeros((2, 2, NTOK, 1024), np.float32)
    for core in range(8):
        b = core // 4
        y[:, b] += results[core]["ypart"]
    return y


def run(inputs, trace=False, **kwargs):
    nc = _get_nc()
    in_maps = _prep(inputs)
    res = run_bass_kernel_spmd(
        nc, in_maps, core_ids=list(range(8)), trace=trace, **kwargs
    )
    return _assemble(res.results), res


def kernel(**inputs) -> np.ndarray:
    y, _ = run(inputs)
    return y



# revision 11
# speedup vs baseline: 1.2248x; 1.2248x over previous
"""Trainium2 Bass kernel for nn_ComplexMultiheadAttention (v2).

Problem: complex multihead attention, B=2, N=1024, D=1024, HEADS=16, d=64.
Sharding (8 cores): core = (b = c//4) x (head group hg = c%4, 4 heads).
Host sums the 4 head-group partial output projections per batch.

v2 changes vs baseline (362.9us):
- all-bf16 matmul dataflow (host pre-casts); fp32 PSUM accumulate.
  Host-sim max rel err ~8e-3 (budget 2e-2).
- all weight DMAs issued up-front on the Scalar HWDGE queue (x on Sync),
  including the phase-C wy weights -> kills the 38.5us pre-C gap and the
  33us serial-DMA head start.
- A/B emission interleaved g-major: exp (the ScalarE pacer, ~147us) starts
  ~45us in and runs near-dense under the PE span (~218us total PE work).
- per-unit normalization: den row broadcast (gpsimd) + reciprocal_approx_fast
  (~5x faster than reciprocal) + one mul; no rp8/rp1 hops.
- phase C evacuations on ScalarE (idle there), DVE freed.
- single shared PSUM slot pool (3x [128,1024] = 6 banks) for A chains,
  attention S tiles, and C chains + 2 banks AV.
"""

import os

import numpy as np
import ml_dtypes

import concourse.mybir as mybir
import concourse.tile as tile
from concourse import bacc
from concourse.bass_utils import run_bass_kernel_spmd

P = 128
NTOK = 1024
KD = 16  # k-tiles over the stacked 2048 contraction dim
CD = 64  # dim per head
HL = 4  # heads per core
F32 = mybir.dt.float32
BF16 = mybir.dt.bfloat16
EXP = mybir.ActivationFunctionType.Exp
SCALE = float(CD) ** -0.5

_nc_cache = None


def _build():
    nc = bacc.Bacc("TRN2", target_bir_lowering=False, debug=False, num_devices=8)

    x = nc.declare_dram_parameter("x", [2048, NTOK], BF16, isOutput=False)
    wnames = ["wqr", "wqi", "wkr", "wki"]
    wd = {n: nc.declare_dram_parameter(n, [2048, 256], BF16, isOutput=False) for n in wnames}
    wv = nc.declare_dram_parameter("wv", [2048, 512], BF16, isOutput=False)
    wyr = nc.declare_dram_parameter("wyr", [512, NTOK], BF16, isOutput=False)
    wyi = nc.declare_dram_parameter("wyi", [512, NTOK], BF16, isOutput=False)
    yp = nc.declare_dram_parameter("ypart", [2, NTOK, 1024], F32, isOutput=True)
    dbg = {}
    if os.environ.get("CMHA_DEBUG"):
        for n, shp, dt in (
            ("d_qrT", [P, HL, NTOK], BF16), ("d_krT", [P, HL, NTOK], BF16),
            ("d_vhat", [P, 8, 2, HL, CD + 1], BF16), ("d_O", [P, HL, NTOK], BF16),
            ("d_pt", [P, 8, 512], BF16), ("d_oav", [CD + 1, NTOK], F32),
        ):
            dbg[n] = nc.declare_dram_parameter(n, shp, dt, isOutput=True)

    with tile.TileContext(nc) as tc:
        with (
            tc.tile_pool(name="persist", bufs=1) as pp,
            tc.tile_pool(name="ps", bufs=3, space="PSUM") as psp,
            tc.tile_pool(name="av", bufs=2, space="PSUM") as avp,
            tc.tile_pool(name="pt", bufs=3) as ptp,
            tc.tile_pool(name="oav", bufs=2) as oavp,
            tc.tile_pool(name="on", bufs=1) as onp,
            tc.tile_pool(name="nrm", bufs=1) as nrmp,
        ):
            # q^T/k^T duplicated along partitions: rows 0:64 and 64:128 both
            # hold head h's 64 dims, so K=128 QK matmuls compute 2*S (factor
            # folded into the exp scale) and keep the PE's HAM monitor armed.
            qrT = pp.tile([P, HL, NTOK], BF16, tag="qrT")
            qiT = pp.tile([P, HL, NTOK], BF16, tag="qiT")
            krT = pp.tile([P, HL, NTOK], BF16, tag="krT")
            kiT = pp.tile([P, HL, NTOK], BF16, tag="kiT")
            # V with ones column appended: [tok-tile, jt, (r,i), head, 65]
            vhat = pp.tile([P, 8, 2, HL, CD + 1], BF16, tag="vhat")
            # combined attention output per head: [or(64); oi(64)] x tokens
            O = pp.tile([P, HL, NTOK], BF16, tag="O")
            wy_sb = pp.tile([P, 2, HL, NTOK], BF16, tag="wy")

            xs = pp.tile([P, KD, NTOK], BF16, tag="xs")

            # ---- all DMAs up front: x on sync, weights on scalar (HWDGE) ----
            xt = x.rearrange("(o p) m -> p o m", p=P)
            for c in range(4):
                nc.sync.dma_start(xs[:, 4 * c : 4 * c + 4, :], xt[:, 4 * c : 4 * c + 4, :])
            wts = {}
            with tc.tile_pool(name="wq", bufs=1) as wqp:
                for n in ("wqr", "wkr", "wqi", "wki"):
                    wts[n] = wqp.tile([P, KD, 256], BF16, tag="wqk", name=f"wt_{n}", bufs=2)
                    nc.scalar.dma_start(
                        wts[n], wd[n].rearrange("(o p) m -> p o m", p=P)
                    )
                    if n == "wkr":
                        wvt = wqp.tile([P, KD, 512], BF16, tag="wt_v")
                        nc.scalar.dma_start(
                            wvt, wv.rearrange("(o p) m -> p o m", p=P)
                        )
                        nc.scalar.dma_start(
                            wy_sb[:, 0], wyr.rearrange("(o p) m -> p o m", p=P)
                        )
                        nc.scalar.dma_start(
                            wy_sb[:, 1], wyi.rearrange("(o p) m -> p o m", p=P)
                        )

                # ---------------- emission helpers ----------------
                def emit_qk_w(wn, dstT):
                    # q^T/k^T = W^T x^T : lhsT = W chunk (stationary), rhs = x
                    for pair in range(2):
                        slot = psp.tile([P, NTOK], F32, tag="s", name="slot")
                        for kt in range(KD):
                            for tch in range(2):
                                nc.tensor.matmul(
                                    slot[:, tch * 512 : (tch + 1) * 512],
                                    wts[wn][:, kt, pair * 128 : (pair + 1) * 128],
                                    xs[:, kt, tch * 512 : (tch + 1) * 512],
                                    start=(kt == 0),
                                    stop=(kt == KD - 1),
                                )
                        hA, hB = pair * 2, pair * 2 + 1
                        nc.vector.tensor_copy(dstT[0:CD, hA, :], slot[0:CD, :])
                        nc.vector.tensor_copy(dstT[CD:P, hB, :], slot[CD:P, :])
                        nc.gpsimd.dma_start(dstT[CD:P, hA, :], dstT[0:CD, hA, :])
                        nc.gpsimd.dma_start(dstT[0:CD, hB, :], dstT[CD:P, hB, :])

                def emit_v(ri):
                    # v = x W : lhsT = x chunk, rhs = wv cols (256 per r/i part)
                    for tt in range(8):
                        slot = psp.tile([P, 256], F32, tag="s", name="slotv")
                        for kt in range(KD):
                            nc.tensor.matmul(
                                slot[:],
                                xs[:, kt, tt * 128 : (tt + 1) * 128],
                                wvt[:, kt, ri * 256 : (ri + 1) * 256],
                                start=(kt == 0),
                                stop=(kt == KD - 1),
                            )
                        nc.vector.tensor_copy(
                            vhat[:, tt, ri, :, 0:CD],
                            slot[:].rearrange("p (h d) -> p h d", d=CD),
                        )
                    nc.vector.memset(vhat[:, :, ri, :, CD : CD + 1], 1.0)

                def emit_hg(h, g, on_h):
                    qT = qrT if g in (0, 1) else qiT
                    kT = krT if g in (0, 2) else kiT
                    vri = 0 if g in (0, 2) else 1
                    oav = oavp.tile([CD + 1, NTOK], F32, tag="oav", name="oav")
                    den0 = nrmp.tile([1, NTOK], F32, tag="den0", name="den0", bufs=1)
                    for ic in range(2):
                        pt = ptp.tile([P, 8, 512], BF16, tag="pt", name="pt")
                        for u in range(4):
                            st = psp.tile([P, NTOK], F32, tag="s", name="st")
                            for jj in range(2):
                                jt = 2 * u + jj
                                nc.tensor.matmul(
                                    st[:, jj * 512 : (jj + 1) * 512],
                                    kT[:, h, jt * 128 : (jt + 1) * 128],
                                    qT[:, h, ic * 512 : (ic + 1) * 512],
                                    start=True,
                                    stop=True,
                                )
                            nc.scalar.activation(
                                pt[:, 2 * u : 2 * u + 2, :].rearrange("p a b -> p (a b)"),
                                st[:],
                                EXP,
                                scale=SCALE / 2.0,
                            )
                        av = avp.tile([CD + 1, 512], F32, tag="av", name="av")
                        for jt in range(8):
                            nc.tensor.matmul(
                                av[:],
                                vhat[:, jt, vri, h, :],
                                pt[:, jt, :],
                                start=(jt == 0),
                                stop=(jt == 7),
                            )
                        icsl = slice(ic * 512, (ic + 1) * 512)
                        if dbg and (h, g, ic) == (0, 0, 0):
                            nc.sync.dma_start(dbg["d_pt"][:, :, :], pt[:])
                        nc.vector.tensor_copy(oav[:, icsl], av[:])
                        nc.gpsimd.dma_start(den0[:, icsl], oav[CD : CD + 1, icsl])
                    # normalization: broadcast den row, reciprocal, multiply
                    if dbg and (h, g) == (0, 0):
                        nc.sync.dma_start(dbg["d_oav"][:, :], oav[:])
                    bcd = nrmp.tile([CD, NTOK], F32, tag="bcd", name="bcd")
                    nc.gpsimd.partition_broadcast(bcd[:], den0[:])
                    bcr = nrmp.tile([CD, NTOK], F32, tag="bcr", name="bcr")
                    nc.vector.reciprocal_approx_fast(bcr[:], bcd[:])
                    nc.vector.tensor_mul(on_h[:, g, :], oav[0:CD, :], bcr[:])

                def emit_combine(h, on_h):
                    # o_r = (o0-o3)-(o1+o2), o_i = (o0-o3)+(o1+o2)
                    # shares the bcd slot: never live at the same time as a bcd
                    sto = nrmp.tile([CD, 2, NTOK], BF16, tag="bcd", name="sto")
                    nc.vector.tensor_sub(sto[:, 0, :], on_h[:, 0, :], on_h[:, 3, :])
                    nc.vector.tensor_add(sto[:, 1, :], on_h[:, 1, :], on_h[:, 2, :])
                    nc.vector.tensor_sub(O[0:CD, h, :], sto[:, 0, :], sto[:, 1, :])
                    # oi scratch shares the bcr slot (bf16 fits in its 4KB)
                    oi = nrmp.tile([CD, NTOK], BF16, tag="bcr", name="oi")
                    nc.vector.tensor_add(oi[:], sto[:, 0, :], sto[:, 1, :])
                    nc.gpsimd.dma_start(O[CD:P, h, :], oi[:])

                on_tiles = {}

                def emit_g_block(g, heads):
                    for h in heads:
                        if h not in on_tiles:
                            on_tiles[h] = onp.tile(
                                [CD, HL, NTOK], BF16, tag=f"on_{h}", name=f"on_{h}"
                            )
                        emit_hg(h, g, on_tiles[h])
                        if g == 3:
                            emit_combine(h, on_tiles[h])

                # ---------------- interleaved A/B emission ----------------
                emit_qk_w("wqr", qrT)
                emit_qk_w("wkr", krT)
                emit_v(0)
                emit_g_block(0, (0, 1, 2, 3))
                emit_qk_w("wqi", qiT)
                emit_g_block(2, (0, 1, 2, 3))
                emit_v(1)
                emit_qk_w("wki", kiT)
                emit_g_block(1, (0, 1, 2, 3))
                emit_g_block(3, (0, 1, 2, 3))

            # ---------------- Phase C: output projection ----------------
            with tc.tile_pool(name="pc_o", bufs=4) as cop:
                for ri in range(2):
                    for tt in range(8):
                        slot = psp.tile([P, NTOK], F32, tag="s", name="sloty")
                        for oc in range(2):
                            for kt in range(HL):
                                nc.tensor.matmul(
                                    slot[:, oc * 512 : (oc + 1) * 512],
                                    O[:, kt, tt * 128 : (tt + 1) * 128],
                                    wy_sb[:, ri, kt, oc * 512 : (oc + 1) * 512],
                                    start=(kt == 0),
                                    stop=(kt == HL - 1),
                                )
                        ys = cop.tile([P, NTOK], F32, tag="ys")
                        nc.scalar.copy(ys[:], slot[:])
                        nc.sync.dma_start(
                            yp[ri, tt * 128 : (tt + 1) * 128, :], ys[:]
                        )
            if dbg:
                nc.sync.dma_start(dbg["d_qrT"][:, :, :], qrT[:])
                nc.sync.dma_start(dbg["d_krT"][:, :, :], krT[:])
                nc.sync.dma_start(dbg["d_vhat"][:, :, :, :, :], vhat[:])
                nc.sync.dma_start(dbg["d_O"][:, :, :], O[:])
    nc.compile()
    return nc


def _prep(inputs):
    f = np.float32
    bf = ml_dtypes.bfloat16
    xr = np.asarray(inputs["x_real"], f)
    xi = np.asarray(inputs["x_imag"], f)
    wq_r = np.asarray(inputs["wq_r"], f)
    wq_i = np.asarray(inputs["wq_i"], f)
    wkv_r = np.asarray(inputs["wkv_r"], f)
    wkv_i = np.asarray(inputs["wkv_i"], f)
    wout_r = np.asarray(inputs["wout_r"], f)
    wout_i = np.asarray(inputs["wout_i"], f)

    c = lambda a: np.ascontiguousarray(a).astype(bf)
    in_maps = []
    for core in range(8):
        b, hg = divmod(core, 4)
        c0 = hg * 256
        X = np.concatenate([xr[b].T, xi[b].T], axis=0)
        sl = slice(c0, c0 + 256)
        vsl = slice(1024 + c0, 1024 + c0 + 256)
        m = {
            "x": c(X),
            "wqr": c(np.concatenate([wq_r[sl].T, -wq_i[sl].T], axis=0)),
            "wqi": c(np.concatenate([wq_i[sl].T, wq_r[sl].T], axis=0)),
            "wkr": c(np.concatenate([wkv_r[sl].T, -wkv_i[sl].T], axis=0)),
            "wki": c(np.concatenate([wkv_i[sl].T, wkv_r[sl].T], axis=0)),
            "wv": c(
                np.concatenate(
                    [
                        np.concatenate([wkv_r[vsl].T, -wkv_i[vsl].T], axis=0),
                        np.concatenate([wkv_i[vsl].T, wkv_r[vsl].T], axis=0),
                    ],
                    axis=1,
                )
            ),
        }
        Wyr = np.empty((512, 1024), f)
        Wyi = np.empty((512, 1024), f)
        for h in range(HL):
            cols = slice(c0 + h * CD, c0 + (h + 1) * CD)
            Wyr[h * 128 : h * 128 + CD] = wout_r[:, cols].T
            Wyr[h * 128 + CD : (h + 1) * 128] = -wout_i[:, cols].T
            Wyi[h * 128 : h * 128 + CD] = wout_i[:, cols].T
            Wyi[h * 128 + CD : (h + 1) * 128] = wout_r[:, cols].T
        m["wyr"] = c(Wyr)
        m["wyi"] = c(Wyi)
        in_maps.append(m)
    return in_maps


def _get_nc():
    global _nc_cache
    if _nc_cache is None:
        _nc_cache = _build()
    return _nc_cache


def _assemble(results):
    y = np.zeros((2, 2, NTOK, 1024), np.float32)
    for core in range(8):
        b = core // 4
        y[:, b] += results[core]["ypart"]
    return y


def run(inputs, trace=False, **kwargs):
    nc = _get_nc()
    in_maps = _prep(inputs)
    res = run_bass_kernel_spmd(
        nc, in_maps, core_ids=list(range(8)), trace=trace, **kwargs
    )
    return _assemble(res.results), res


def kernel(**inputs) -> np.ndarray:
    y, _ = run(inputs)
    return y


# revision 12
# speedup vs baseline: 1.2257x; 1.0007x over previous
"""Trainium2 Bass kernel for nn_ComplexMultiheadAttention (v2).

Problem: complex multihead attention, B=2, N=1024, D=1024, HEADS=16, d=64.
Sharding (8 cores): core = (b = c//4) x (head group hg = c%4, 4 heads).
Host sums the 4 head-group partial output projections per batch.

v2 changes vs baseline (362.9us):
- all-bf16 matmul dataflow (host pre-casts); fp32 PSUM accumulate.
  Host-sim max rel err ~8e-3 (budget 2e-2).
- all weight DMAs issued up-front on the Scalar HWDGE queue (x on Sync),
  including the phase-C wy weights -> kills the 38.5us pre-C gap and the
  33us serial-DMA head start.
- A/B emission interleaved g-major: exp (the ScalarE pacer, ~147us) starts
  ~45us in and runs near-dense under the PE span (~218us total PE work).
- per-unit normalization: den row broadcast (gpsimd) + reciprocal_approx_fast
  (~5x faster than reciprocal) + one mul; no rp8/rp1 hops.
- phase C evacuations on ScalarE (idle there), DVE freed.
- single shared PSUM slot pool (3x [128,1024] = 6 banks) for A chains,
  attention S tiles, and C chains + 2 banks AV.
"""

import os

import numpy as np
import ml_dtypes

import concourse.mybir as mybir
import concourse.tile as tile
from concourse import bacc
from concourse.bass_utils import run_bass_kernel_spmd

P = 128
NTOK = 1024
KD = 16  # k-tiles over the stacked 2048 contraction dim
CD = 64  # dim per head
HL = 4  # heads per core
F32 = mybir.dt.float32
BF16 = mybir.dt.bfloat16
EXP = mybir.ActivationFunctionType.Exp
SCALE = float(CD) ** -0.5

_nc_cache = None


def _build():
    nc = bacc.Bacc("TRN2", target_bir_lowering=False, debug=False, num_devices=8)

    x = nc.declare_dram_parameter("x", [2048, NTOK], BF16, isOutput=False)
    wnames = ["wqr", "wqi", "wkr", "wki"]
    wd = {n: nc.declare_dram_parameter(n, [2048, 256], BF16, isOutput=False) for n in wnames}
    wv = nc.declare_dram_parameter("wv", [2048, 512], BF16, isOutput=False)
    wyr = nc.declare_dram_parameter("wyr", [512, NTOK], BF16, isOutput=False)
    wyi = nc.declare_dram_parameter("wyi", [512, NTOK], BF16, isOutput=False)
    yp = nc.declare_dram_parameter("ypart", [2, NTOK, 1024], F32, isOutput=True)
    dbg = {}
    if os.environ.get("CMHA_DEBUG"):
        for n, shp, dt in (
            ("d_qrT", [P, HL, NTOK], BF16), ("d_krT", [P, HL, NTOK], BF16),
            ("d_vhat", [P, 8, 2, HL, CD + 1], BF16), ("d_O", [P, HL, NTOK], BF16),
            ("d_pt", [P, 8, 512], BF16), ("d_oav", [CD + 1, NTOK], F32),
        ):
            dbg[n] = nc.declare_dram_parameter(n, shp, dt, isOutput=True)

    with tile.TileContext(nc) as tc:
        with (
            tc.tile_pool(name="persist", bufs=1) as pp,
            tc.tile_pool(name="ps", bufs=3, space="PSUM") as psp,
            tc.tile_pool(name="av", bufs=2, space="PSUM") as avp,
            tc.tile_pool(name="pt", bufs=3) as ptp,
            tc.tile_pool(name="oav", bufs=2) as oavp,
            tc.tile_pool(name="on", bufs=1) as onp,
            tc.tile_pool(name="nrm", bufs=1) as nrmp,
        ):
            # q^T/k^T duplicated along partitions: rows 0:64 and 64:128 both
            # hold head h's 64 dims, so K=128 QK matmuls compute 2*S (factor
            # folded into the exp scale) and keep the PE's HAM monitor armed.
            qrT = pp.tile([P, HL, NTOK], BF16, tag="qrT")
            qiT = pp.tile([P, HL, NTOK], BF16, tag="qiT")
            krT = pp.tile([P, HL, NTOK], BF16, tag="krT")
            kiT = pp.tile([P, HL, NTOK], BF16, tag="kiT")
            # V with ones column appended: [tok-tile, jt, (r,i), head, 65]
            vhat = pp.tile([P, 8, 2, HL, CD + 1], BF16, tag="vhat")
            # combined attention output per head: [or(64); oi(64)] x tokens
            O = pp.tile([P, HL, NTOK], BF16, tag="O")
            wy_sb = pp.tile([P, 2, HL, NTOK], BF16, tag="wy")

            xs = pp.tile([P, KD, NTOK], BF16, tag="xs")

            # ---- all DMAs up front: x on sync, weights on scalar (HWDGE) ----
            xt = x.rearrange("(o p) m -> p o m", p=P)
            for lo, hi in ((0, 2), (2, 4), (4, 8), (8, 12), (12, 16)):
                nc.sync.dma_start(xs[:, lo:hi, :], xt[:, lo:hi, :])
            wts = {}
            with tc.tile_pool(name="wq", bufs=1) as wqp:
                for n in ("wqr", "wkr", "wqi", "wki"):
                    wts[n] = wqp.tile([P, KD, 256], BF16, tag="wqk", name=f"wt_{n}", bufs=2)
                    wsrc = wd[n].rearrange("(o p) m -> p o m", p=P)
                    if n in ("wqr", "wkr"):
                        nc.scalar.dma_start(wts[n][:, :, 0:128], wsrc[:, :, 0:128])
                        nc.scalar.dma_start(wts[n][:, :, 128:256], wsrc[:, :, 128:256])
                    else:
                        nc.scalar.dma_start(wts[n], wsrc)
                    if n == "wkr":
                        wvt = wqp.tile([P, KD, 512], BF16, tag="wt_v")
                        nc.scalar.dma_start(
                            wvt, wv.rearrange("(o p) m -> p o m", p=P)
                        )
                        nc.scalar.dma_start(
                            wy_sb[:, 0], wyr.rearrange("(o p) m -> p o m", p=P)
                        )
                        nc.scalar.dma_start(
                            wy_sb[:, 1], wyi.rearrange("(o p) m -> p o m", p=P)
                        )

                # ---------------- emission helpers ----------------
                def emit_qk_w(wn, dstT):
                    # q^T/k^T = W^T x^T : lhsT = W chunk (stationary), rhs = x
                    for pair in range(2):
                        slot = psp.tile([P, NTOK], F32, tag="s", name="slot")
                        for kt in range(KD):
                            for tch in range(2):
                                nc.tensor.matmul(
                                    slot[:, tch * 512 : (tch + 1) * 512],
                                    wts[wn][:, kt, pair * 128 : (pair + 1) * 128],
                                    xs[:, kt, tch * 512 : (tch + 1) * 512],
                                    start=(kt == 0),
                                    stop=(kt == KD - 1),
                                )
                        hA, hB = pair * 2, pair * 2 + 1
                        nc.vector.tensor_copy(dstT[0:CD, hA, :], slot[0:CD, :])
                        nc.vector.tensor_copy(dstT[CD:P, hB, :], slot[CD:P, :])
                        nc.gpsimd.dma_start(dstT[CD:P, hA, :], dstT[0:CD, hA, :])
                        nc.gpsimd.dma_start(dstT[0:CD, hB, :], dstT[CD:P, hB, :])

                def emit_v(ri):
                    # v = x W : lhsT = x chunk, rhs = wv cols (256 per r/i part)
                    for tt in range(8):
                        slot = psp.tile([P, 256], F32, tag="s", name="slotv")
                        for kt in range(KD):
                            nc.tensor.matmul(
                                slot[:],
                                xs[:, kt, tt * 128 : (tt + 1) * 128],
                                wvt[:, kt, ri * 256 : (ri + 1) * 256],
                                start=(kt == 0),
                                stop=(kt == KD - 1),
                            )
                        nc.vector.tensor_copy(
                            vhat[:, tt, ri, :, 0:CD],
                            slot[:].rearrange("p (h d) -> p h d", d=CD),
                        )
                    nc.vector.memset(vhat[:, :, ri, :, CD : CD + 1], 1.0)

                def emit_qk_exp(h, g, ic):
                    qT = qrT if g in (0, 1) else qiT
                    kT = krT if g in (0, 2) else kiT
                    pt = ptp.tile([P, 8, 512], BF16, tag="pt", name="pt")
                    for u in range(4):
                        st = psp.tile([P, NTOK], F32, tag="s", name="st")
                        for jj in range(2):
                            jt = 2 * u + jj
                            nc.tensor.matmul(
                                st[:, jj * 512 : (jj + 1) * 512],
                                kT[:, h, jt * 128 : (jt + 1) * 128],
                                qT[:, h, ic * 512 : (ic + 1) * 512],
                                start=True,
                                stop=True,
                            )
                        nc.scalar.activation(
                            pt[:, 2 * u : 2 * u + 2, :].rearrange("p a b -> p (a b)"),
                            st[:],
                            EXP,
                            scale=SCALE / 2.0,
                        )
                    return pt

                def emit_hg(h, g, on_h, pre=None):
                    vri = 0 if g in (0, 2) else 1
                    oav = oavp.tile([CD + 1, NTOK], F32, tag="oav", name="oav")
                    den0 = nrmp.tile([1, NTOK], F32, tag="den0", name="den0", bufs=1)
                    for ic in range(2):
                        if pre is not None and ic in pre:
                            pt = pre[ic]
                        else:
                            pt = emit_qk_exp(h, g, ic)
                        av = avp.tile([CD + 1, 512], F32, tag="av", name="av")
                        for jt in range(8):
                            nc.tensor.matmul(
                                av[:],
                                vhat[:, jt, vri, h, :],
                                pt[:, jt, :],
                                start=(jt == 0),
                                stop=(jt == 7),
                            )
                        icsl = slice(ic * 512, (ic + 1) * 512)
                        if dbg and (h, g, ic) == (0, 0, 0):
                            nc.sync.dma_start(dbg["d_pt"][:, :, :], pt[:])
                        nc.vector.tensor_copy(oav[:, icsl], av[:])
                        nc.gpsimd.dma_start(den0[:, icsl], oav[CD : CD + 1, icsl])
                    if dbg and (h, g) == (0, 0):
                        nc.sync.dma_start(dbg["d_oav"][:, :], oav[:])
                    # normalization: broadcast den row, reciprocal, multiply
                    bcd = nrmp.tile([CD, NTOK], F32, tag="bcd", name="bcd")
                    nc.gpsimd.partition_broadcast(bcd[:], den0[:])
                    bcr = nrmp.tile([CD, NTOK], F32, tag="bcr", name="bcr")
                    nc.vector.reciprocal_approx_fast(bcr[:], bcd[:])
                    nc.vector.tensor_mul(on_h[:, g, :], oav[0:CD, :], bcr[:])

                def emit_combine(h, on_h):
                    # o_r = (o0-o3)-(o1+o2), o_i = (o0-o3)+(o1+o2)
                    # shares the bcd slot: never live at the same time as a bcd
                    sto = nrmp.tile([CD, 2, NTOK], BF16, tag="bcd", name="sto")
                    nc.vector.tensor_sub(sto[:, 0, :], on_h[:, 0, :], on_h[:, 3, :])
                    nc.vector.tensor_add(sto[:, 1, :], on_h[:, 1, :], on_h[:, 2, :])
                    nc.vector.tensor_sub(O[0:CD, h, :], sto[:, 0, :], sto[:, 1, :])
                    # oi scratch shares the bcr slot (bf16 fits in its 4KB)
                    oi = nrmp.tile([CD, NTOK], BF16, tag="bcr", name="oi")
                    nc.vector.tensor_add(oi[:], sto[:, 0, :], sto[:, 1, :])
                    nc.gpsimd.dma_start(O[CD:P, h, :], oi[:])

                on_tiles = {}

                def emit_g_block(g, heads, pre=None):
                    for h in heads:
                        if h not in on_tiles:
                            on_tiles[h] = onp.tile(
                                [CD, HL, NTOK], BF16, tag=f"on_{h}", name=f"on_{h}"
                            )
                        emit_hg(h, g, on_tiles[h], (pre or {}).get(h))
                        if g == 3:
                            emit_combine(h, on_tiles[h])

                # ---------------- interleaved A/B emission ----------------
                emit_qk_w("wqr", qrT)
                emit_qk_w("wkr", krT)
                # exp warm-up: ScalarE gets work while the v chains run on PE
                warm = {0: {0: emit_qk_exp(0, 0, 0), 1: emit_qk_exp(0, 0, 1)},
                        1: {0: emit_qk_exp(1, 0, 0)}}
                emit_v(0)
                emit_g_block(0, (0, 1, 2, 3), pre=warm)
                emit_qk_w("wqi", qiT)
                emit_g_block(2, (0, 1, 2, 3))
                emit_v(1)
                emit_qk_w("wki", kiT)
                emit_g_block(1, (0, 1, 2, 3))
                emit_g_block(3, (0, 1, 2, 3))

            # ---------------- Phase C: output projection ----------------
            with tc.tile_pool(name="pc_o", bufs=4) as cop:
                for ri in range(2):
                    for tt in range(8):
                        slot = psp.tile([P, NTOK], F32, tag="s", name="sloty")
                        for oc in range(2):
                            for kt in range(HL):
                                nc.tensor.matmul(
                                    slot[:, oc * 512 : (oc + 1) * 512],
                                    O[:, kt, tt * 128 : (tt + 1) * 128],
                                    wy_sb[:, ri, kt, oc * 512 : (oc + 1) * 512],
                                    start=(kt == 0),
                                    stop=(kt == HL - 1),
                                )
                        ys = cop.tile([P, NTOK], F32, tag="ys")
                        nc.scalar.copy(ys[:], slot[:])
                        nc.sync.dma_start(
                            yp[ri, tt * 128 : (tt + 1) * 128, :], ys[:]
                        )
            if dbg:
                nc.sync.dma_start(dbg["d_qrT"][:, :, :], qrT[:])
                nc.sync.dma_start(dbg["d_krT"][:, :, :], krT[:])
                nc.sync.dma_start(dbg["d_vhat"][:, :, :, :, :], vhat[:])
                nc.sync.dma_start(dbg["d_O"][:, :, :], O[:])
    nc.compile()
    return nc


def _prep(inputs):
    f = np.float32
    bf = ml_dtypes.bfloat16
    xr = np.asarray(inputs["x_real"], f)
    xi = np.asarray(inputs["x_imag"], f)
    wq_r = np.asarray(inputs["wq_r"], f)
    wq_i = np.asarray(inputs["wq_i"], f)
    wkv_r = np.asarray(inputs["wkv_r"], f)
    wkv_i = np.asarray(inputs["wkv_i"], f)
    wout_r = np.asarray(inputs["wout_r"], f)
    wout_i = np.asarray(inputs["wout_i"], f)

    c = lambda a: np.ascontiguousarray(a).astype(bf)
    in_maps = []
    for core in range(8):
        b, hg = divmod(core, 4)
        c0 = hg * 256
        X = np.concatenate([xr[b].T, xi[b].T], axis=0)
        sl = slice(c0, c0 + 256)
        vsl = slice(1024 + c0, 1024 + c0 + 256)
        m = {
            "x": c(X),
            "wqr": c(np.concatenate([wq_r[sl].T, -wq_i[sl].T], axis=0)),
            "wqi": c(np.concatenate([wq_i[sl].T, wq_r[sl].T], axis=0)),
            "wkr": c(np.concatenate([wkv_r[sl].T, -wkv_i[sl].T], axis=0)),
            "wki": c(np.concatenate([wkv_i[sl].T, wkv_r[sl].T], axis=0)),
            "wv": c(
                np.concatenate(
                    [
                        np.concatenate([wkv_r[vsl].T, -wkv_i[vsl].T], axis=0),
                        np.concatenate([wkv_i[vsl].T, wkv_r[vsl].T], axis=0),
                    ],
                    axis=1,
                )
            ),
        }
        Wyr = np.empty((512, 1024), f)
        Wyi = np.empty((512, 1024), f)
        for h in range(HL):
            cols = slice(c0 + h * CD, c0 + (h + 1) * CD)
            Wyr[h * 128 : h * 128 + CD] = wout_r[:, cols].T
            Wyr[h * 128 + CD : (h + 1) * 128] = -wout_i[:, cols].T
            Wyi[h * 128 : h * 128 + CD] = wout_i[:, cols].T
            Wyi[h * 128 + CD : (h + 1) * 128] = wout_r[:, cols].T
        m["wyr"] = c(Wyr)
        m["wyi"] = c(Wyi)
        in_maps.append(m)
    return in_maps


def _get_nc():
    global _nc_cache
    if _nc_cache is None:
        _nc_cache = _build()
    return _nc_cache


def _assemble(results):
    y = np.zeros((2, 2, NTOK, 1024), np.float32)
    for core in range(8):
        b = core // 4
        y[:, b] += results[core]["ypart"]
    return y


def run(inputs, trace=False, **kwargs):
    nc = _get_nc()
    in_maps = _prep(inputs)
    res = run_bass_kernel_spmd(
        nc, in_maps, core_ids=list(range(8)), trace=trace, **kwargs
    )
    return _assemble(res.results), res


def kernel(**inputs) -> np.ndarray:
    y, _ = run(inputs)
    return y
